# revision 31
# baseline (speedup 1.0000x reference)
"""CapsuleLayer (dynamic routing, 3 iterations) on 8 Trainium2 NeuronCores.

v11: v2 + (a) agreement d-fold moved from DVE to PE via psum-accumulated
identity-stationary matmuls, (b) fold-residual PSUM pull on the Scalar
engine, (c) fp16 b-logits, (d) pre-replicated vv broadcast for the
agree-mult, (e) segmented den-AllReduce (quarters for iteration 0,
halves for iteration 1) with input/output DMAs split so they never
head-of-line block each other, (f) W-prefetch DMAs moved to the
Activation HWDGE ring so their pacing semaphores can't stall the
collective traffic on the SP ring, (g) all production PSUM->SBUF
copies on Scalar to keep the DVE routing spine clean.

Math (see reference):
    x_hat[b,o,i,d] = sum_m W[o,i,d,m] * x[b,i,m]
    b_log = 0; for it in 0..2:
        c = softmax(b_log, axis=o)
        s = sum_i c[b,o,i] * x_hat[b,o,i,d]; out = squash(s)
        if it < 2: b_log += x_hat . out

Why O-sharding: every reduction except the softmax denominator is local.
  - iteration 0 (uniform c): s0 = (1/32) sum_i x_hat  -> local (no AR!)
  - agreement b += x_hat . out                        -> local
  - weighted sum s = sum_i c x_hat                    -> local
  - softmax over o: den[b,i] = sum_o exp(b)           -> AllReduce (128KB f32)
The den-AR is split in halves and overlaps the second half of the
agreement pass; weight DMA stays 16MB/core (same as I-sharding).

Per-core layout: partition = (pi, b) [pi = a half-index, b = batch].
x_hat in SBUF fp16 as 16 chunk tiles [128, 16j x 256], free = (j, d, o_l)
with d-major, o_l-minor (q = d*4 + o): the c/vv broadcasts keep innermost
stride-1 (DVE 2x eligible).

The agreement pass sum_d x_hat[b,o,i,d]*out[b,o,d]: DVE does the
broadcast multiply (t = xh .* vv), then the d-reduction runs on the PE:
8 accumulating matmuls per chunk (512-col movs hide LDWEIGHTS) with an
eye(128) stationary, each summing an 8-wide d-block into
psum[(pi,b), (j, dA, o)].  Scalar pulls the psum residual to SBUF; DVE
folds the final 8->1.  This removes the
log-tree d-fold (~40% of DVE work) from the critical DVE spine and keeps
the PE warm (HAM stays at K=8/8) through the agreement phases.

i-sums run on PE with a [128,128] kron(ones(2,2), eye(64)) stationary
that contracts (pi,b) while keeping b diagonal and replicating the result
into both partition halves, with a 2-step psum merge at the end.
"""

import time

import numpy as np

import concourse.bacc as bacc
import concourse.mybir as mybir
import concourse.tile as tile

B, O, I, D, M = 64, 32, 512, 64, 64
CORES = 8
OL = O // CORES          # 4 output capsules per core
J = I // 2               # 256 i-pairs per core (full I resident)
Q = OL * D               # 256 free elems per i-pair: q = d*OL + o
NCHUNK = 16              # x_hat chunk tiles
JC = J // NCHUNK         # 16 i-pairs per chunk
NWT = 32                 # weight DMA tiles (8 j each)
EPS = 1e-8

F16 = mybir.dt.float16
F32 = mybir.dt.float32


def _build(repeat=1, stage=7):
    nc = bacc.Bacc("TRN2", target_bir_lowering=False, debug=False,
                   num_devices=CORES)
    ALU = mybir.AluOpType
    AX = mybir.AxisListType.X

    xt_d = nc.dram_tensor("xt", [128, J * B], F16, kind="ExternalInput").ap()
    wt_d = nc.dram_tensor("wt", [NWT, 128, (J // NWT) * Q], F16,
                          kind="ExternalInput").ap()
    dls_d = nc.dram_tensor("dls", [128, 128], F16, kind="ExternalInput").ap()
    dlw_d = nc.dram_tensor("dlw", [128, 128], F16, kind="ExternalInput").ap()
    dle_d = nc.dram_tensor("dle", [128, 128], F16, kind="ExternalInput").ap()
    out_d = nc.dram_tensor("outp", [64, Q], F32, kind="ExternalOutput").ap()

    with tile.TileContext(nc) as tc:
        with (
            tc.tile_pool(name="big", bufs=1) as big,
            tc.tile_pool(name="wp", bufs=3) as wp,
            tc.tile_pool(name="scr", bufs=2) as scr,
            tc.tile_pool(name="small", bufs=1) as small,
            tc.tile_pool(name="stats", bufs=1) as stats,
            tc.tile_pool(name="ppool", bufs=2, space="PSUM") as ppool,
            tc.tile_pool(name="spool", bufs=2, space="PSUM") as spool,
            tc.tile_pool(name="fpool", bufs=2, space="PSUM") as fpool,
            tc.tile_pool(name="dram", bufs=1, space="DRAM") as dram,
        ):
            xall = small.tile([128, J * B], F16, tag="xall")
            dls = small.tile([128, 128], F16, tag="dls")
            dlw = small.tile([128, 128], F16, tag="dlw")
            dle = small.tile([128, 128], F16, tag="dle")
            nc.sync.dma_start(xall[:], xt_d)
            nc.sync.dma_start(dls[:], dls_d)
            nc.sync.dma_start(dlw[:], dlw_d)
            nc.sync.dma_start(dle[:], dle_d)

            xh = [big.tile([128, JC * Q], F16, tag=f"xh{g}", name=f"xh{g}")
                  for g in range(NCHUNK)]
            vv = small.tile([128, Q], F16, tag="vv", bufs=2)
            vvb = small.tile([128, 4 * Q], F16, tag="vvb")
            b1 = small.tile([128, J * OL], F16, tag="b1")
            ee = small.tile([128, J * OL], F32, tag="ee")
            cs = small.tile([128, J * OL], F16, tag="cs")
            den = small.tile([128, J], F32, tag="den")
            deng = small.tile([128, J], F32, tag="deng")
            ob = small.tile([64, Q], F32, tag="ob")
            ss = small.tile([128, Q], F32, tag="ss", bufs=2)

            for rep in range(repeat):
                # ---- production + dsum (iteration-0 i-sum), interleaved ----
                s0p = spool.tile([128, 2 * Q], F32, tag="sp")

                def dsum_wtile(w):
                    # 8 j's of wtile w -> 4 accumulating matmuls (2 j each)
                    g, half = (w * 8) // JC, (w * 8) % JC
                    src = xh[g][:, half * Q:(half + 8) * Q]
                    for m2 in range(4):
                        nc.tensor.matmul(
                            s0p[:, :],
                            dls[:],
                            src[:, m2 * 2 * Q:(m2 + 1) * 2 * Q],
                            start=(w == 0 and m2 == 0),
                            stop=(w == NWT - 1 and m2 == 3),
                        )

                for w in range(NWT):
                    wj = wp.tile([128, 8 * Q], F16, tag="w")
                    # W loads go on the Activation HWDGE ring: their pacing
                    # semaphores (wp buffer recycling) would otherwise
                    # head-of-line block the AR collective DMAs on the SP
                    # ring, serializing the softmax-denominator exchange.
                    nc.scalar.dma_start(wj[:], wt_d[w])
                    for hf in range(2):      # two psum tiles of 4 j each
                        pt = ppool.tile([128, 4 * Q], F32, tag="pt")
                        for jj in range(4):
                            j = w * 8 + hf * 4 + jj
                            for i2 in range(2):
                                # 2-quadrant packing: (0,0)+(64,64) pairs
                                # stream concurrently (cross-quadrant
                                # (0,64)/(64,0) faults at runtime on trn2)
                                pi = i2
                                sl = slice(i2 * 64, (i2 + 1) * 64)
                                ol = slice(pi * 64, (pi + 1) * 64)
                                nc.tensor.matmul(
                                    pt[ol, jj * Q:(jj + 1) * Q],
                                    xall[sl, j * B:(j + 1) * B],
                                    wj[sl, (hf * 4 + jj) * Q:
                                           (hf * 4 + jj + 1) * Q],
                                    start=True, stop=True,
                                    tile_position=(i2 * 64, pi * 64),
                                )
                        g = (w * 8) // JC
                        off = ((w * 8) % JC + hf * 4) * Q
                        dst = xh[g][:, off:off + 4 * Q]
                        # all copies on Scalar: keeps the DVE routing spine
                        # free (Tile would otherwise schedule these casts
                        # into the ws-pass AR-wait slots and overshoot)
                        nc.scalar.copy(dst, pt[:])
                    if w >= 3:
                        dsum_wtile(w - 3)
                for w in range(NWT - 3, NWT):
                    dsum_wtile(w)

                def psum_merge(sp, dst):
                    # sum the 2 j-parity psum partials into SBUF dst
                    # (a DVE op may read at most one PSUM operand)
                    nc.scalar.copy(dst[:], sp[:, Q:2 * Q])
                    nc.vector.tensor_add(dst[:], dst[:], sp[:, 0:Q])

                # ---- squash: factor f = n2/((1+n2)(n+eps)), n2 = sum_d s^2
                def squash(sv, out_ap, out_f32):
                    sq = stats.tile([128, Q], F32, tag="sq")
                    nc.vector.tensor_mul(sq[:], sv[:], sv[:])
                    n2 = stats.tile([128, OL], F32, tag="n2")
                    nc.vector.reduce_sum(
                        n2[:], sq.rearrange("p (d o) -> p o d", o=OL), axis=AX)
                    n1 = stats.tile([128, OL], F32, tag="n1")
                    nc.scalar.sqrt(n1[:], n2[:])
                    t1 = stats.tile([128, OL], F32, tag="t1")
                    nc.vector.tensor_scalar_add(t1[:], n2[:], 1.0)
                    nc.vector.reciprocal(t1[:], t1[:])
                    t2 = stats.tile([128, OL], F32, tag="t2")
                    nc.vector.tensor_scalar_add(t2[:], n1[:], EPS)
                    nc.vector.reciprocal(t2[:], t2[:])
                    ff = stats.tile([128, OL], F32, tag="ff")
                    nc.vector.tensor_mul(ff[:], n2[:], t1[:])
                    f2 = stats.tile([128, OL], F32, tag="f2")
                    nc.vector.tensor_mul(f2[:], ff[:], t2[:])
                    if out_f32:
                        nc.vector.tensor_tensor(
                            out_ap.rearrange("p (d o) -> p d o", o=OL),
                            sv[0:64, :].rearrange("p (d o) -> p d o", o=OL),
                            f2[0:64].unsqueeze(1).broadcast_to([64, D, OL]),
                            ALU.mult,
                        )
                    else:
                        f2h = stats.tile([128, OL], F16, tag="f2h")
                        nc.vector.tensor_copy(f2h[:], f2[:])
                        nc.vector.tensor_tensor(
                            out_ap.rearrange("p (d o) -> p d o", o=OL),
                            sv[:].rearrange("p (d o) -> p d o", o=OL),
                            f2h.unsqueeze(1).broadcast_to([128, D, OL]),
                            ALU.mult,
                        )

                # ---- one routing iteration ----------------------------------
                def agree_chunk(g, last_iter):
                    t = scr.tile([128, JC * Q], F16, tag="t")
                    nc.vector.tensor_tensor(
                        t[:],
                        xh[g][:],
                        vvb.unsqueeze(1).broadcast_to([128, 4, 4 * Q]),
                        ALU.mult,
                    )
                    # fold d 64 -> 8 on the PE: 8 accumulating matmuls
                    # (eye stationary), each one 8-wide d-block (32 q cols).
                    fp = fpool.tile([128, JC * 32], F32, tag="fp")
                    t4 = t.rearrange("p (j dB q) -> p j dB q", dB=8, q=32)
                    for dB in range(8):
                        nc.tensor.matmul(
                            fp[:], dle[:], t4[:, :, dB, :],
                            start=(dB == 0), stop=(dB == 7),
                        )
                    # residual fold 8 -> 1: scalar pulls psum, DVE adds
                    u = stats.tile([128, JC * 32], F16, tag="u")
                    nc.scalar.copy(u[:], fp[:])
                    u3 = u.rearrange("p (j x) -> p j x", x=32)
                    nc.vector.tensor_add(u3[:, :, 0:16],
                                         u3[:, :, 0:16], u3[:, :, 16:32])
                    nc.vector.tensor_add(u3[:, :, 0:8],
                                         u3[:, :, 0:8], u3[:, :, 8:16])
                    bsl = b1[:, g * JC * OL:(g + 1) * JC * OL]
                    b3 = bsl.rearrange("p (j o) -> p j o", o=OL)
                    if not last_iter:
                        nc.vector.tensor_add(b3, u3[:, :, 0:4],
                                             u3[:, :, 4:8])
                    else:
                        nc.vector.tensor_add(u3[:, :, 0:4], u3[:, :, 0:4],
                                             u3[:, :, 4:8])
                        nc.vector.tensor_add(b3, b3, u3[:, :, 0:4])

                def exp_den_kick2(s, seg, ar_in, ar_out):
                    # softmax numerator + local denominator for a segment
                    # of seg chunks, then kick the AllReduce of den.  The
                    # result pull (deng) is emitted separately (deng_pull2)
                    # so it doesn't head-of-line block later segments'
                    # input DMAs on the sync queue.
                    nj = seg * JC
                    sl = slice(s * nj * OL, (s + 1) * nj * OL)
                    dsl = slice(s * nj, (s + 1) * nj)
                    nc.scalar.activation(ee[:, sl], b1[:, sl],
                                         mybir.ActivationFunctionType.Exp)
                    nc.vector.reduce_sum(
                        den[:, dsl],
                        ee[:, sl].rearrange("p (j o) -> p j o", o=OL),
                        axis=AX)
                    nc.sync.dma_start(ar_in[:], den[:, dsl])
                    nc.gpsimd.collective_compute(
                        "AllReduce",
                        ALU.add,
                        replica_groups=[list(range(CORES))],
                        ins=[ar_in.opt()],
                        outs=[ar_out.opt()],
                    )

                def deng_pull2(s, seg, ar_out):
                    nj = seg * JC
                    dsl = slice(s * nj, (s + 1) * nj)
                    nc.sync.dma_start(deng[:, dsl], ar_out[:])

                def recip_c2(s, seg):
                    nj = seg * JC
                    dsl = slice(s * nj, (s + 1) * nj)
                    sl = slice(s * nj * OL, (s + 1) * nj * OL)
                    nc.vector.reciprocal_approx_fast(deng[:, dsl],
                                                     deng[:, dsl])
                    nc.vector.tensor_tensor(
                        cs[:, sl].rearrange("p (j o) -> p j o", o=OL),
                        ee[:, sl].rearrange("p (j o) -> p j o", o=OL),
                        deng[:, dsl].unsqueeze(2).broadcast_to(
                            [128, nj, OL]),
                        ALU.mult,
                    )

                def ws_chunk(g, sp):
                    xc = scr.tile([128, JC * Q], F16, tag="t")
                    nc.vector.tensor_tensor(
                        xc.rearrange("p (j d o) -> p j d o", d=D, o=OL),
                        xh[g].rearrange("p (j d o) -> p j d o", d=D, o=OL),
                        cs[:, g * JC * OL:(g + 1) * JC * OL]
                        .rearrange("p (j o) -> p j o", o=OL)
                        .unsqueeze(2).broadcast_to([128, JC, D, OL]),
                        ALU.mult,
                    )
                    for m2 in range(8):
                        nc.tensor.matmul(
                            sp[:, :],
                            dlw[:],
                            xc[:, m2 * 2 * Q:(m2 + 1) * 2 * Q],
                            start=(g == 0 and m2 == 0),
                            stop=(g == NCHUNK - 1 and m2 == 7),
                        )

                # ================= routing =================
                # iteration 0: c uniform (1/32 baked into dls) -> local s0
                s0s = ss  # alias (pool rotates by tag)
                psum_merge(s0p, s0s)
                if stage == 1:
                    nc.vector.tensor_copy(ob[:], s0s[0:64, :])
                    nc.sync.dma_start(out_d, ob[:])
                    continue
                def build_vvb():
                    # replicate vv 4x so the agree-mult broadcast AP has a
                    # 4-long outer loop instead of 16 (less DVE AP overhead)
                    nc.vector.tensor_copy(
                        vvb.rearrange("p (r q) -> p r q", q=Q),
                        vv.unsqueeze(1).broadcast_to([128, 4, Q]))
                    # HAM warmer: the squash boundary idles the PE >3.4us,
                    # re-throttling it to 1.2GHz for the whole agreement
                    # pass (fold matmuls then gate the DVE spine).  A short
                    # vv-dependent dummy-matmul burst holds K=8/8 across
                    # the boundary; result consumed cheaply to defeat DCE.
                    ptd = ppool.tile([128, 4 * Q], F32, tag="pt")
                    for k in range(12):
                        nc.tensor.matmul(ptd[:, 0:Q], dle[:], vv[:],
                                         start=(k == 0), stop=(k == 11))
                    wrm = stats.tile([128, 16], F32, tag="wrm")
                    nc.scalar.copy(wrm[:], ptd[:, 0:16])

                squash(s0s, vv[:], out_f32=False)       # vv = out0
                build_vvb()
                if stage == 2:
                    nc.vector.tensor_copy(ob[:], vv[0:64, :])
                    nc.sync.dma_start(out_d, ob[:])
                    continue

                ar_bufs = []
                for it in range(2):
                    nseg = 4 if it == 0 else 2
                    for h in range(nseg):
                        ai = dram.tile([128, J // nseg], F32,
                                       tag=f"ari_{rep}_{it}_{h}",
                                       name=f"ari_{rep}_{it}_{h}")
                        ao = dram.tile([128, J // nseg], F32,
                                       tag=f"aro_{rep}_{it}_{h}",
                                       name=f"aro_{rep}_{it}_{h}",
                                       addr_space="Shared")
                        ar_bufs.append((ai, ao))

                def iteration(it, sp, last):
                    # it0: quarter ARs (hide fully behind remaining agree
                    # chunks + ws1).  it1: the CC paces ~20-25us percollective
                    # during the W-prefetch of the next rep, so fewer,
                    # larger segments stall ws2 less.
                    seg = 4 if it == 0 else 8
                    nseg = NCHUNK // seg
                    for g in range(NCHUNK):
                        agree_chunk(g, last_iter=(it == 1))
                        if g % seg == seg - 1:
                            exp_den_kick2(g // seg, seg,
                                          *ar_bufs[4 * it + g // seg])
                    for g in range(NCHUNK):
                        if g % seg == 0:
                            deng_pull2(g // seg, seg,
                                       ar_bufs[4 * it + g // seg][1])
                            recip_c2(g // seg, seg)
                        ws_chunk(g, sp)

                # iteration 1
                s1p = spool.tile([128, 2 * Q], F32, tag="sp")
                iteration(0, s1p, last=False)
                s1s = small.tile([128, Q], F32, tag="ss", bufs=2, name="s1s")
                psum_merge(s1p, s1s)
                if stage == 3:
                    nc.vector.tensor_copy(ob[:], b1[0:64, 0:Q])
                    nc.sync.dma_start(out_d, ob[:])
                    continue
                if stage == 4:
                    nc.vector.tensor_copy(ob[:], den[0:64, 0:J][:, 0:Q])
                    nc.sync.dma_start(out_d, ob[:])
                    continue
                if stage == 5:
                    nc.vector.tensor_copy(ob[:], cs[0:64, 0:Q])
                    nc.sync.dma_start(out_d, ob[:])
                    continue
                if stage == 6:
                    nc.vector.tensor_copy(ob[:], s1s[0:64, :])
                    nc.sync.dma_start(out_d, ob[:])
                    continue
                squash(s1s, vv[:], out_f32=False)       # vv = out1
                build_vvb()

                # iteration 2
                s2p = spool.tile([128, 2 * Q], F32, tag="sp")
                iteration(1, s2p, last=True)
                s2s = small.tile([128, Q], F32, tag="ss", bufs=2, name="s2s")
                psum_merge(s2p, s2s)
                squash(s2s, ob[:], out_f32=True)        # ob = out2 (f32)
                nc.sync.dma_start(out_d, ob[:])

    nc.compile()
    return nc


def _prep(x, weight):
    """Host-side shard + relayout + fp16 cast."""
    x16 = x.astype(np.float16)
    w16 = weight.astype(np.float16)
    # x: [B, I, M] -> [i2, m, j, b] -> [128, J*B]   (i = 2j + i2)
    xc = x16.reshape(B, J, 2, M).transpose(2, 3, 1, 0)
    xall = np.ascontiguousarray(xc.reshape(128, J * B))
    ws = []
    for c in range(CORES):
        wc = w16[c * OL:(c + 1) * OL]                 # [OL, I, D, M]
        # [o, j, i2, d, m] -> [j, i2, m, d, o]
        wc = wc.reshape(OL, J, 2, D, M).transpose(1, 2, 4, 3, 0)
        # group 8 j per wtile: [NWT, 8, 128, Q] -> [NWT, 128, 8*Q]
        wc = wc.reshape(NWT, 8, 128, Q).transpose(0, 2, 1, 3)
        ws.append(np.ascontiguousarray(wc.reshape(NWT, 128, 8 * Q)))
    kr = np.kron(np.ones((2, 2), np.float16), np.eye(64, dtype=np.float16))
    dls = kr / np.float16(32.0)
    dlw = kr
    dle = np.eye(128, dtype=np.float16)
    return xall, ws, dls, dlw, dle


class _Runner:
    """Compile once, execute many times (same as v1)."""

    def __init__(self, nc):
        import jax
        from jax.sharding import Mesh, PartitionSpec
        from jax.experimental.shard_map import shard_map
        from concourse import bass2jax
        from concourse.bass2jax import install_neuronx_cc_hook

        install_neuronx_cc_hook()
        self.nc = nc
        partition_name = (nc.partition_id_tensor.name
                          if nc.partition_id_tensor else None)
        in_names, out_names, out_avals, zero_outs = [], [], [], []
        for alloc in nc.m.functions[0].allocations:
            if not isinstance(alloc, mybir.MemoryLocationSet):
                continue
            name = alloc.memorylocations[0].name
            if alloc.kind == "ExternalInput":
                if name != partition_name:
                    in_names.append(name)
            elif alloc.kind == "ExternalOutput":
                out_names.append(name)
                shape = tuple(alloc.tensor_shape)
                dtype = mybir.dt.np(alloc.dtype)
                out_avals.append(jax.core.ShapedArray(shape, dtype))
                zero_outs.append(np.zeros(shape, dtype))
        n_params = len(in_names)
        n_outs = len(out_avals)
        all_in_names = list(in_names) + list(out_names)
        if partition_name is not None:
            all_in_names.append(partition_name)
        self.in_names = in_names
        self.out_names = out_names
        self.zero_outs = zero_outs
        self.out_avals = out_avals

        def _body(*args):
            operands = list(args)
            if partition_name is not None:
                operands.append(bass2jax.partition_id_tensor())
            outs = bass2jax._bass_exec_p.bind(
                *operands,
                out_avals=tuple(out_avals),
                in_names=tuple(all_in_names),
                out_names=tuple(out_names),
                lowering_input_output_aliases=(),
                sim_require_finite=True,
                sim_require_nnan=True,
                nc=nc,
            )
            return tuple(outs)

        devices = jax.devices()[:CORES]
        assert len(devices) == CORES
        mesh = Mesh(np.asarray(devices), ("core",))
        in_specs = (PartitionSpec("core"),) * (n_params + n_outs)
        out_specs = (PartitionSpec("core"),) * n_outs
        donate = tuple(range(n_params, n_params + n_outs))
        self.sharded = jax.jit(
            shard_map(_body, mesh=mesh, in_specs=in_specs,
                      out_specs=out_specs, check_rep=False),
            donate_argnums=donate, keep_unused=True,
        )

    def __call__(self, in_maps):
        concat_in = [
            np.concatenate([np.asarray(m[name]) for m in in_maps], axis=0)
            for name in self.in_names
        ]
        concat_zeros = [
            np.zeros((CORES * z.shape[0], *z.shape[1:]), z.dtype)
            for z in self.zero_outs
        ]
        out_arrs = self.sharded(*concat_in, *concat_zeros)
        return [
            {
                name: np.asarray(out_arrs[i]).reshape(
                    CORES, *self.out_avals[i].shape)[c]
                for i, name in enumerate(self.out_names)
            }
            for c in range(CORES)
        ]


_RUNNERS = {}


def _get_runner(repeat=1, stage=7):
    key = (repeat, stage)
    if key not in _RUNNERS:
        _RUNNERS[key] = _Runner(_build(repeat, stage))
    return _RUNNERS[key]


def make_in_maps(x, weight):
    xall, ws, dls, dlw, dle = _prep(np.asarray(x, np.float32),
                                    np.asarray(weight, np.float32))
    return [{"xt": xall, "wt": ws[c], "dls": dls, "dlw": dlw, "dle": dle}
            for c in range(CORES)]


def finish(results):
    # outp per core: [B=64, Q=256] f32 with q = d*OL + o_l
    parts = []
    for c in range(CORES):
        oc = results[c]["outp"].reshape(B, D, OL).transpose(0, 2, 1)
        parts.append(oc)                              # [B, OL, D]
    return np.ascontiguousarray(np.concatenate(parts, axis=1),
                                dtype=np.float32)     # [B, O, D]


def kernel(x, weight):
    runner = _get_runner()
    results = runner(make_in_maps(x, weight))
    return finish(results)


if __name__ == "__main__":
    rng = np.random.default_rng(0)
    x = rng.standard_normal((B, I, M)).astype(np.float32)
    w = (rng.standard_normal((O, I, D, M)) * 0.1).astype(np.float32)
    t0 = time.time()
    out = kernel(x, w)
    print("first call (incl compile):", time.time() - t0, "s; out", out.shape)

    # host reference
    xh = np.einsum("oidm,bim->boid", w.astype(np.float32), x)
    bb = np.zeros((B, O, I), np.float32)

    def squash_np(v):
        n = np.linalg.norm(v, axis=-1, keepdims=True)
        return (n * n / (1 + n * n)) * v / (n + EPS)

    for it in range(3):
        e = np.exp(bb - bb.max(axis=1, keepdims=True))
        c = e / e.sum(axis=1, keepdims=True)
        s = np.einsum("boi,boid->bod", c, xh)
        o = squash_np(s)
        if it < 2:
            bb = bb + np.einsum("boid,bod->boi", xh, o)
    err = np.abs(out - o).max() / (np.abs(o).max() + 1e-12)
    print("rel err vs host reference:", err)



# revision 32
# speedup vs baseline: 1.1377x; 1.1377x over previous
"""CapsuleLayer (dynamic routing, 3 iterations) on 8 Trainium2 NeuronCores.

v11: v2 + (a) agreement d-fold moved from DVE to PE via psum-accumulated
identity-stationary matmuls, (b) fold-residual PSUM pull on the Scalar
engine, (c) fp16 b-logits, (d) pre-replicated vv broadcast for the
agree-mult, (e) segmented den-AllReduce (quarters for iteration 0,
halves for iteration 1) with input/output DMAs split so they never
head-of-line block each other, (f) W-prefetch DMAs moved to the
Activation HWDGE ring so their pacing semaphores can't stall the
collective traffic on the SP ring, (g) all production PSUM->SBUF
copies on Scalar to keep the DVE routing spine clean.

Math (see reference):
    x_hat[b,o,i,d] = sum_m W[o,i,d,m] * x[b,i,m]
    b_log = 0; for it in 0..2:
        c = softmax(b_log, axis=o)
        s = sum_i c[b,o,i] * x_hat[b,o,i,d]; out = squash(s)
        if it < 2: b_log += x_hat . out

Why O-sharding: every reduction except the softmax denominator is local.
  - iteration 0 (uniform c): s0 = (1/32) sum_i x_hat  -> local (no AR!)
  - agreement b += x_hat . out                        -> local
  - weighted sum s = sum_i c x_hat                    -> local
  - softmax over o: den[b,i] = sum_o exp(b)           -> AllReduce (128KB f32)
The den-AR is split in halves and overlaps the second half of the
agreement pass; weight DMA stays 16MB/core (same as I-sharding).

Per-core layout: partition = (pi, b) [pi = a half-index, b = batch].
x_hat in SBUF fp16 as 16 chunk tiles [128, 16j x 256], free = (j, d, o_l)
with d-major, o_l-minor (q = d*4 + o): the c/vv broadcasts keep innermost
stride-1 (DVE 2x eligible).

The agreement pass sum_d x_hat[b,o,i,d]*out[b,o,d]: DVE does the
broadcast multiply (t = xh .* vv), then the d-reduction runs on the PE:
8 accumulating matmuls per chunk (512-col movs hide LDWEIGHTS) with an
eye(128) stationary, each summing an 8-wide d-block into
psum[(pi,b), (j, dA, o)].  Scalar pulls the psum residual to SBUF; DVE
folds the final 8->1.  This removes the
log-tree d-fold (~40% of DVE work) from the critical DVE spine and keeps
the PE warm (HAM stays at K=8/8) through the agreement phases.

i-sums run on PE with a [128,128] kron(ones(2,2), eye(64)) stationary
that contracts (pi,b) while keeping b diagonal and replicating the result
into both partition halves, with a 2-step psum merge at the end.
"""

import time

import numpy as np

import concourse.bacc as bacc
import concourse.mybir as mybir
import concourse.tile as tile

B, O, I, D, M = 64, 32, 512, 64, 64
CORES = 8
OL = O // CORES          # 4 output capsules per core
J = I // 2               # 256 i-pairs per core (full I resident)
Q = OL * D               # 256 free elems per i-pair: q = d*OL + o
NCHUNK = 16              # x_hat chunk tiles
JC = J // NCHUNK         # 16 i-pairs per chunk
NWT = 32                 # weight DMA tiles (8 j each)
EPS = 1e-8

F16 = mybir.dt.float16
F32 = mybir.dt.float32


def _build(repeat=1, stage=7):
    nc = bacc.Bacc("TRN2", target_bir_lowering=False, debug=False,
                   num_devices=CORES)
    ALU = mybir.AluOpType
    AX = mybir.AxisListType.X

    xt_d = nc.dram_tensor("xt", [128, J * B], F16, kind="ExternalInput").ap()
    wt_d = nc.dram_tensor("wt", [NWT, 128, (J // NWT) * Q], F16,
                          kind="ExternalInput").ap()
    dls_d = nc.dram_tensor("dls", [128, 128], F16, kind="ExternalInput").ap()
    dlw_d = nc.dram_tensor("dlw", [128, 128], F16, kind="ExternalInput").ap()
    dle_d = nc.dram_tensor("dle", [128, 128], F16, kind="ExternalInput").ap()
    out_d = nc.dram_tensor("outp", [64, Q], F32, kind="ExternalOutput").ap()

    with tile.TileContext(nc) as tc:
        with (
            tc.tile_pool(name="big", bufs=1) as big,
            tc.tile_pool(name="wp", bufs=3) as wp,
            tc.tile_pool(name="scr", bufs=2) as scr,
            tc.tile_pool(name="small", bufs=1) as small,
            tc.tile_pool(name="stats", bufs=1) as stats,
            tc.tile_pool(name="ppool", bufs=2, space="PSUM") as ppool,
            tc.tile_pool(name="spool", bufs=2, space="PSUM") as spool,
            tc.tile_pool(name="fpool", bufs=2, space="PSUM") as fpool,
            tc.tile_pool(name="dram", bufs=1, space="DRAM") as dram,
        ):
            xall = small.tile([128, J * B], F16, tag="xall")
            dls = small.tile([128, 128], F16, tag="dls")
            dlw = small.tile([128, 128], F16, tag="dlw")
            dle = small.tile([128, 128], F16, tag="dle")
            nc.sync.dma_start(xall[:], xt_d)
            nc.sync.dma_start(dls[:], dls_d)
            nc.sync.dma_start(dlw[:], dlw_d)
            nc.sync.dma_start(dle[:], dle_d)

            xh = [big.tile([128, JC * Q], F16, tag=f"xh{g}", name=f"xh{g}")
                  for g in range(NCHUNK)]
            vv = small.tile([128, Q], F16, tag="vv", bufs=2)
            vvb = small.tile([128, 4 * Q], F16, tag="vvb")
            b1 = small.tile([128, J * OL], F16, tag="b1")
            ee = small.tile([128, J * OL], F32, tag="ee")
            cs = small.tile([128, J * OL], F16, tag="cs")
            den = small.tile([128, J], F32, tag="den")
            deng = small.tile([128, J], F32, tag="deng")
            ob = small.tile([64, Q], F32, tag="ob")
            ss = small.tile([128, Q], F32, tag="ss", bufs=2)

            for rep in range(repeat):
                # ---- production + dsum (iteration-0 i-sum), interleaved ----
                s0p = spool.tile([128, 2 * Q], F32, tag="sp")

                def dsum_wtile(w):
                    # 8 j's of wtile w -> 4 accumulating matmuls (2 j each)
                    g, half = (w * 8) // JC, (w * 8) % JC
                    src = xh[g][:, half * Q:(half + 8) * Q]
                    for m2 in range(4):
                        nc.tensor.matmul(
                            s0p[:, :],
                            dls[:],
                            src[:, m2 * 2 * Q:(m2 + 1) * 2 * Q],
                            start=(w == 0 and m2 == 0),
                            stop=(w == NWT - 1 and m2 == 3),
                        )

                for w in range(NWT):
                    wj = wp.tile([128, 8 * Q], F16, tag="w")
                    # W loads go on the Activation HWDGE ring: their pacing
                    # semaphores (wp buffer recycling) would otherwise
                    # head-of-line block the AR collective DMAs on the SP
                    # ring, serializing the softmax-denominator exchange.
                    nc.scalar.dma_start(wj[:], wt_d[w])
                    for hf in range(2):      # two psum tiles of 4 j each
                        pt = ppool.tile([128, 4 * Q], F32, tag="pt")
                        for jj in range(4):
                            j = w * 8 + hf * 4 + jj
                            for i2 in range(2):
                                # 2-quadrant packing: (0,0)+(64,64) pairs
                                # stream concurrently (cross-quadrant
                                # (0,64)/(64,0) faults at runtime on trn2)
                                pi = i2
                                sl = slice(i2 * 64, (i2 + 1) * 64)
                                ol = slice(pi * 64, (pi + 1) * 64)
                                nc.tensor.matmul(
                                    pt[ol, jj * Q:(jj + 1) * Q],
                                    xall[sl, j * B:(j + 1) * B],
                                    wj[sl, (hf * 4 + jj) * Q:
                                           (hf * 4 + jj + 1) * Q],
                                    start=True, stop=True,
                                    tile_position=(i2 * 64, pi * 64),
                                )
                        g = (w * 8) // JC
                        off = ((w * 8) % JC + hf * 4) * Q
                        dst = xh[g][:, off:off + 4 * Q]
                        # all copies on Scalar: keeps the DVE routing spine
                        # free (Tile would otherwise schedule these casts
                        # into the ws-pass AR-wait slots and overshoot)
                        nc.scalar.copy(dst, pt[:])
                    if w >= 3:
                        dsum_wtile(w - 3)
                for w in range(NWT - 3, NWT):
                    dsum_wtile(w)

                def psum_merge(sp, dst):
                    # sum the 2 j-parity psum partials into SBUF dst
                    # (a DVE op may read at most one PSUM operand)
                    nc.scalar.copy(dst[:], sp[:, Q:2 * Q])
                    nc.vector.tensor_add(dst[:], dst[:], sp[:, 0:Q])

                # ---- squash: factor f = n2/((1+n2)(n+eps)), n2 = sum_d s^2
                def squash(sv, out_ap, out_f32):
                    sq = stats.tile([128, Q], F32, tag="sq")
                    nc.vector.tensor_mul(sq[:], sv[:], sv[:])
                    n2 = stats.tile([128, OL], F32, tag="n2")
                    nc.vector.reduce_sum(
                        n2[:], sq.rearrange("p (d o) -> p o d", o=OL), axis=AX)
                    n1 = stats.tile([128, OL], F32, tag="n1")
                    nc.scalar.sqrt(n1[:], n2[:])
                    t1 = stats.tile([128, OL], F32, tag="t1")
                    nc.vector.tensor_scalar_add(t1[:], n2[:], 1.0)
                    nc.vector.reciprocal(t1[:], t1[:])
                    t2 = stats.tile([128, OL], F32, tag="t2")
                    nc.vector.tensor_scalar_add(t2[:], n1[:], EPS)
                    nc.vector.reciprocal(t2[:], t2[:])
                    ff = stats.tile([128, OL], F32, tag="ff")
                    nc.vector.tensor_mul(ff[:], n2[:], t1[:])
                    f2 = stats.tile([128, OL], F32, tag="f2")
                    nc.vector.tensor_mul(f2[:], ff[:], t2[:])
                    if out_f32:
                        nc.vector.tensor_tensor(
                            out_ap.rearrange("p (d o) -> p d o", o=OL),
                            sv[0:64, :].rearrange("p (d o) -> p d o", o=OL),
                            f2[0:64].unsqueeze(1).broadcast_to([64, D, OL]),
                            ALU.mult,
                        )
                    else:
                        f2h = stats.tile([128, OL], F16, tag="f2h")
                        nc.vector.tensor_copy(f2h[:], f2[:])
                        nc.vector.tensor_tensor(
                            out_ap.rearrange("p (d o) -> p d o", o=OL),
                            sv[:].rearrange("p (d o) -> p d o", o=OL),
                            f2h.unsqueeze(1).broadcast_to([128, D, OL]),
                            ALU.mult,
                        )

                # ---- one routing iteration ----------------------------------
                def agree_chunk(g, last_iter):
                    t = scr.tile([128, JC * Q], F16, tag="t")
                    nc.vector.tensor_tensor(
                        t[:],
                        xh[g][:],
                        vvb.unsqueeze(1).broadcast_to([128, 4, 4 * Q]),
                        ALU.mult,
                    )
                    # fold d 64 -> 8 on the PE: 8 accumulating matmuls
                    # (eye stationary), each one 8-wide d-block (32 q cols).
                    fp = fpool.tile([128, JC * 32], F32, tag="fp")
                    t4 = t.rearrange("p (j dB q) -> p j dB q", dB=8, q=32)
                    for dB in range(8):
                        nc.tensor.matmul(
                            fp[:], dle[:], t4[:, :, dB, :],
                            start=(dB == 0), stop=(dB == 7),
                        )
                    # residual fold 8 -> 1: scalar pulls psum, DVE adds
                    u = stats.tile([128, JC * 32], F16, tag="u")
                    nc.scalar.copy(u[:], fp[:])
                    u3 = u.rearrange("p (j x) -> p j x", x=32)
                    nc.vector.tensor_add(u3[:, :, 0:16],
                                         u3[:, :, 0:16], u3[:, :, 16:32])
                    nc.vector.tensor_add(u3[:, :, 0:8],
                                         u3[:, :, 0:8], u3[:, :, 8:16])
                    bsl = b1[:, g * JC * OL:(g + 1) * JC * OL]
                    b3 = bsl.rearrange("p (j o) -> p j o", o=OL)
                    if not last_iter:
                        nc.vector.tensor_add(b3, u3[:, :, 0:4],
                                             u3[:, :, 4:8])
                    else:
                        nc.vector.tensor_add(u3[:, :, 0:4], u3[:, :, 0:4],
                                             u3[:, :, 4:8])
                        nc.vector.tensor_add(b3, b3, u3[:, :, 0:4])

                def exp_den_kick2(s, seg, ar_in, ar_out):
                    # softmax numerator + local denominator for a segment
                    # of seg chunks, then kick the AllReduce of den.  The
                    # result pull (deng) is emitted separately (deng_pull2)
                    # so it doesn't head-of-line block later segments'
                    # input DMAs on the sync queue.
                    nj = seg * JC
                    sl = slice(s * nj * OL, (s + 1) * nj * OL)
                    dsl = slice(s * nj, (s + 1) * nj)
                    nc.scalar.activation(ee[:, sl], b1[:, sl],
                                         mybir.ActivationFunctionType.Exp)
                    nc.vector.reduce_sum(
                        den[:, dsl],
                        ee[:, sl].rearrange("p (j o) -> p j o", o=OL),
                        axis=AX)
                    nc.sync.dma_start(ar_in[:], den[:, dsl])
                    nc.gpsimd.collective_compute(
                        "AllReduce",
                        ALU.add,
                        replica_groups=[list(range(CORES))],
                        ins=[ar_in.opt()],
                        outs=[ar_out.opt()],
                    )

                def deng_pull2(s, seg, ar_out):
                    nj = seg * JC
                    dsl = slice(s * nj, (s + 1) * nj)
                    nc.sync.dma_start(deng[:, dsl], ar_out[:])

                def recip_c2(s, seg):
                    nj = seg * JC
                    dsl = slice(s * nj, (s + 1) * nj)
                    sl = slice(s * nj * OL, (s + 1) * nj * OL)
                    nc.vector.reciprocal_approx_fast(deng[:, dsl],
                                                     deng[:, dsl])
                    nc.vector.tensor_tensor(
                        cs[:, sl].rearrange("p (j o) -> p j o", o=OL),
                        ee[:, sl].rearrange("p (j o) -> p j o", o=OL),
                        deng[:, dsl].unsqueeze(2).broadcast_to(
                            [128, nj, OL]),
                        ALU.mult,
                    )

                def ws_chunk(g, sp):
                    xc = scr.tile([128, JC * Q], F16, tag="t")
                    nc.vector.tensor_tensor(
                        xc.rearrange("p (j d o) -> p j d o", d=D, o=OL),
                        xh[g].rearrange("p (j d o) -> p j d o", d=D, o=OL),
                        cs[:, g * JC * OL:(g + 1) * JC * OL]
                        .rearrange("p (j o) -> p j o", o=OL)
                        .unsqueeze(2).broadcast_to([128, JC, D, OL]),
                        ALU.mult,
                    )
                    for m2 in range(8):
                        nc.tensor.matmul(
                            sp[:, :],
                            dlw[:],
                            xc[:, m2 * 2 * Q:(m2 + 1) * 2 * Q],
                            start=(g == 0 and m2 == 0),
                            stop=(g == NCHUNK - 1 and m2 == 7),
                        )

                # ================= routing =================
                # iteration 0: c uniform (1/32 baked into dls) -> local s0
                s0s = ss  # alias (pool rotates by tag)
                psum_merge(s0p, s0s)
                if stage == 1:
                    nc.vector.tensor_copy(ob[:], s0s[0:64, :])
                    nc.sync.dma_start(out_d, ob[:])
                    continue
                def build_vvb():
                    # replicate vv 4x so the agree-mult broadcast AP has a
                    # 4-long outer loop instead of 16 (less DVE AP overhead)
                    nc.vector.tensor_copy(
                        vvb.rearrange("p (r q) -> p r q", q=Q),
                        vv.unsqueeze(1).broadcast_to([128, 4, Q]))

                squash(s0s, vv[:], out_f32=False)       # vv = out0
                build_vvb()
                if stage == 2:
                    nc.vector.tensor_copy(ob[:], vv[0:64, :])
                    nc.sync.dma_start(out_d, ob[:])
                    continue

                ar_bufs = []
                for it in range(2):
                    nseg = 4 if it == 0 else 2
                    for h in range(nseg):
                        ai = dram.tile([128, J // nseg], F32,
                                       tag=f"ari_{rep}_{it}_{h}",
                                       name=f"ari_{rep}_{it}_{h}")
                        ao = dram.tile([128, J // nseg], F32,
                                       tag=f"aro_{rep}_{it}_{h}",
                                       name=f"aro_{rep}_{it}_{h}",
                                       addr_space="Shared")
                        ar_bufs.append((ai, ao))

                def iteration(it, sp, last):
                    # it0: quarter ARs (hide fully behind remaining agree
                    # chunks + ws1).  it1: the CC paces ~20-25us percollective
                    # during the W-prefetch of the next rep, so fewer,
                    # larger segments stall ws2 less.
                    seg = 4 if it == 0 else 8
                    nseg = NCHUNK // seg
                    for g in range(NCHUNK):
                        agree_chunk(g, last_iter=(it == 1))
                        if g % seg == seg - 1:
                            exp_den_kick2(g // seg, seg,
                                          *ar_bufs[4 * it + g // seg])
                    for g in range(NCHUNK):
                        if g % seg == 0:
                            deng_pull2(g // seg, seg,
                                       ar_bufs[4 * it + g // seg][1])
                            recip_c2(g // seg, seg)
                        ws_chunk(g, sp)

                # iteration 1
                s1p = spool.tile([128, 2 * Q], F32, tag="sp")
                iteration(0, s1p, last=False)
                s1s = small.tile([128, Q], F32, tag="ss", bufs=2, name="s1s")
                psum_merge(s1p, s1s)
                if stage == 3:
                    nc.vector.tensor_copy(ob[:], b1[0:64, 0:Q])
                    nc.sync.dma_start(out_d, ob[:])
                    continue
                if stage == 4:
                    nc.vector.tensor_copy(ob[:], den[0:64, 0:J][:, 0:Q])
                    nc.sync.dma_start(out_d, ob[:])
                    continue
                if stage == 5:
                    nc.vector.tensor_copy(ob[:], cs[0:64, 0:Q])
                    nc.sync.dma_start(out_d, ob[:])
                    continue
                if stage == 6:
                    nc.vector.tensor_copy(ob[:], s1s[0:64, :])
                    nc.sync.dma_start(out_d, ob[:])
                    continue
                squash(s1s, vv[:], out_f32=False)       # vv = out1
                build_vvb()

                # iteration 2
                s2p = spool.tile([128, 2 * Q], F32, tag="sp")
                iteration(1, s2p, last=True)
                s2s = small.tile([128, Q], F32, tag="ss", bufs=2, name="s2s")
                psum_merge(s2p, s2s)
                squash(s2s, ob[:], out_f32=True)        # ob = out2 (f32)
                nc.sync.dma_start(out_d, ob[:])

    nc.compile()
    return nc


def _prep(x, weight):
    """Host-side shard + relayout + fp16 cast."""
    x16 = x.astype(np.float16)
    w16 = weight.astype(np.float16)
    # x: [B, I, M] -> [i2, m, j, b] -> [128, J*B]   (i = 2j + i2)
    xc = x16.reshape(B, J, 2, M).transpose(2, 3, 1, 0)
    xall = np.ascontiguousarray(xc.reshape(128, J * B))
    ws = []
    for c in range(CORES):
        wc = w16[c * OL:(c + 1) * OL]                 # [OL, I, D, M]
        # [o, j, i2, d, m] -> [j, i2, m, d, o]
        wc = wc.reshape(OL, J, 2, D, M).transpose(1, 2, 4, 3, 0)
        # group 8 j per wtile: [NWT, 8, 128, Q] -> [NWT, 128, 8*Q]
        wc = wc.reshape(NWT, 8, 128, Q).transpose(0, 2, 1, 3)
        ws.append(np.ascontiguousarray(wc.reshape(NWT, 128, 8 * Q)))
    kr = np.kron(np.ones((2, 2), np.float16), np.eye(64, dtype=np.float16))
    dls = kr / np.float16(32.0)
    dlw = kr
    dle = np.eye(128, dtype=np.float16)
    return xall, ws, dls, dlw, dle


class _Runner:
    """Compile once, execute many times (same as v1)."""

    def __init__(self, nc):
        import jax
        from jax.sharding import Mesh, PartitionSpec
        from jax.experimental.shard_map import shard_map
        from concourse import bass2jax
        from concourse.bass2jax import install_neuronx_cc_hook

        install_neuronx_cc_hook()
        self.nc = nc
        partition_name = (nc.partition_id_tensor.name
                          if nc.partition_id_tensor else None)
        in_names, out_names, out_avals, zero_outs = [], [], [], []
        for alloc in nc.m.functions[0].allocations:
            if not isinstance(alloc, mybir.MemoryLocationSet):
                continue
            name = alloc.memorylocations[0].name
            if alloc.kind == "ExternalInput":
                if name != partition_name:
                    in_names.append(name)
            elif alloc.kind == "ExternalOutput":
                out_names.append(name)
                shape = tuple(alloc.tensor_shape)
                dtype = mybir.dt.np(alloc.dtype)
                out_avals.append(jax.core.ShapedArray(shape, dtype))
                zero_outs.append(np.zeros(shape, dtype))
        n_params = len(in_names)
        n_outs = len(out_avals)
        all_in_names = list(in_names) + list(out_names)
        if partition_name is not None:
            all_in_names.append(partition_name)
        self.in_names = in_names
        self.out_names = out_names
        self.zero_outs = zero_outs
        self.out_avals = out_avals

        def _body(*args):
            operands = list(args)
            if partition_name is not None:
                operands.append(bass2jax.partition_id_tensor())
            outs = bass2jax._bass_exec_p.bind(
                *operands,
                out_avals=tuple(out_avals),
                in_names=tuple(all_in_names),
                out_names=tuple(out_names),
                lowering_input_output_aliases=(),
                sim_require_finite=True,
                sim_require_nnan=True,
                nc=nc,
            )
            return tuple(outs)

        devices = jax.devices()[:CORES]
        assert len(devices) == CORES
        mesh = Mesh(np.asarray(devices), ("core",))
        in_specs = (PartitionSpec("core"),) * (n_params + n_outs)
        out_specs = (PartitionSpec("core"),) * n_outs
        donate = tuple(range(n_params, n_params + n_outs))
        self.sharded = jax.jit(
            shard_map(_body, mesh=mesh, in_specs=in_specs,
                      out_specs=out_specs, check_rep=False),
            donate_argnums=donate, keep_unused=True,
        )

    def __call__(self, in_maps):
        concat_in = [
            np.concatenate([np.asarray(m[name]) for m in in_maps], axis=0)
            for name in self.in_names
        ]
        concat_zeros = [
            np.zeros((CORES * z.shape[0], *z.shape[1:]), z.dtype)
            for z in self.zero_outs
        ]
        out_arrs = self.sharded(*concat_in, *concat_zeros)
        return [
            {
                name: np.asarray(out_arrs[i]).reshape(
                    CORES, *self.out_avals[i].shape)[c]
                for i, name in enumerate(self.out_names)
            }
            for c in range(CORES)
        ]


_RUNNERS = {}


def _get_runner(repeat=1, stage=7):
    key = (repeat, stage)
    if key not in _RUNNERS:
        _RUNNERS[key] = _Runner(_build(repeat, stage))
    return _RUNNERS[key]


def make_in_maps(x, weight):
    xall, ws, dls, dlw, dle = _prep(np.asarray(x, np.float32),
                                    np.asarray(weight, np.float32))
    return [{"xt": xall, "wt": ws[c], "dls": dls, "dlw": dlw, "dle": dle}
            for c in range(CORES)]


def finish(results):
    # outp per core: [B=64, Q=256] f32 with q = d*OL + o_l
    parts = []
    for c in range(CORES):
        oc = results[c]["outp"].reshape(B, D, OL).transpose(0, 2, 1)
        parts.append(oc)                              # [B, OL, D]
    return np.ascontiguousarray(np.concatenate(parts, axis=1),
                                dtype=np.float32)     # [B, O, D]


def kernel(x, weight):
    runner = _get_runner()
    results = runner(make_in_maps(x, weight))
    return finish(results)


if __name__ == "__main__":
    rng = np.random.default_rng(0)
    x = rng.standard_normal((B, I, M)).astype(np.float32)
    w = (rng.standard_normal((O, I, D, M)) * 0.1).astype(np.float32)
    t0 = time.time()
    out = kernel(x, w)
    print("first call (incl compile):", time.time() - t0, "s; out", out.shape)

    # host reference
    xh = np.einsum("oidm,bim->boid", w.astype(np.float32), x)
    bb = np.zeros((B, O, I), np.float32)

    def squash_np(v):
        n = np.linalg.norm(v, axis=-1, keepdims=True)
        return (n * n / (1 + n * n)) * v / (n + EPS)

    for it in range(3):
        e = np.exp(bb - bb.max(axis=1, keepdims=True))
        c = e / e.sum(axis=1, keepdims=True)
        s = np.einsum("boi,boid->bod", c, xh)
        o = squash_np(s)
        if it < 2:
            bb = bb + np.einsum("boid,bod->boi", xh, o)
    err = np.abs(out - o).max() / (np.abs(o).max() + 1e-12)
    print("rel err vs host reference:", err)



# revision 33
# speedup vs baseline: 1.1987x; 1.0536x over previous
"""CapsuleLayer (dynamic routing, 3 iterations) on 8 Trainium2 NeuronCores.

v11: v2 + (a) agreement d-fold moved from DVE to PE via psum-accumulated
identity-stationary matmuls, (b) fold-residual PSUM pull on the Scalar
engine, (c) fp16 b-logits, (d) pre-replicated vv broadcast for the
agree-mult, (e) segmented den-AllReduce (quarters for iteration 0,
halves for iteration 1) with input/output DMAs split so they never
head-of-line block each other, (f) W-prefetch DMAs moved to the
Activation HWDGE ring so their pacing semaphores can't stall the
collective traffic on the SP ring, (g) all production PSUM->SBUF
copies on Scalar to keep the DVE routing spine clean.

Math (see reference):
    x_hat[b,o,i,d] = sum_m W[o,i,d,m] * x[b,i,m]
    b_log = 0; for it in 0..2:
        c = softmax(b_log, axis=o)
        s = sum_i c[b,o,i] * x_hat[b,o,i,d]; out = squash(s)
        if it < 2: b_log += x_hat . out

Why O-sharding: every reduction except the softmax denominator is local.
  - iteration 0 (uniform c): s0 = (1/32) sum_i x_hat  -> local (no AR!)
  - agreement b += x_hat . out                        -> local
  - weighted sum s = sum_i c x_hat                    -> local
  - softmax over o: den[b,i] = sum_o exp(b)           -> AllReduce (128KB f32)
The den-AR is split in halves and overlaps the second half of the
agreement pass; weight DMA stays 16MB/core (same as I-sharding).

Per-core layout: partition = (pi, b) [pi = a half-index, b = batch].
x_hat in SBUF fp16 as 16 chunk tiles [128, 16j x 256], free = (j, d, o_l)
with d-major, o_l-minor (q = d*4 + o): the c/vv broadcasts keep innermost
stride-1 (DVE 2x eligible).

The agreement pass sum_d x_hat[b,o,i,d]*out[b,o,d]: DVE does the
broadcast multiply (t = xh .* vv), then the d-reduction runs on the PE:
8 accumulating matmuls per chunk (512-col movs hide LDWEIGHTS) with an
eye(128) stationary, each summing an 8-wide d-block into
psum[(pi,b), (j, dA, o)].  Scalar pulls the psum residual to SBUF; DVE
folds the final 8->1.  This removes the
log-tree d-fold (~40% of DVE work) from the critical DVE spine and keeps
the PE warm (HAM stays at K=8/8) through the agreement phases.

i-sums run on PE with a [128,128] kron(ones(2,2), eye(64)) stationary
that contracts (pi,b) while keeping b diagonal and replicating the result
into both partition halves, with a 2-step psum merge at the end.
"""

import time

import numpy as np

import concourse.bacc as bacc
import concourse.mybir as mybir
import concourse.tile as tile

B, O, I, D, M = 64, 32, 512, 64, 64
CORES = 8
OL = O // CORES          # 4 output capsules per core
J = I // 2               # 256 i-pairs per core (full I resident)
Q = OL * D               # 256 free elems per i-pair: q = d*OL + o
NCHUNK = 16              # x_hat chunk tiles
JC = J // NCHUNK         # 16 i-pairs per chunk
NWT = 32                 # weight DMA tiles (8 j each)
EPS = 1e-8

F16 = mybir.dt.float16
F32 = mybir.dt.float32


def _build(repeat=1, stage=7):
    nc = bacc.Bacc("TRN2", target_bir_lowering=False, debug=False,
                   num_devices=CORES)
    ALU = mybir.AluOpType
    AX = mybir.AxisListType.X

    xt_d = nc.dram_tensor("xt", [128, J * B], F16, kind="ExternalInput").ap()
    wt_d = nc.dram_tensor("wt", [NWT, 128, (J // NWT) * Q], F16,
                          kind="ExternalInput").ap()
    dls_d = nc.dram_tensor("dls", [128, 128], F16, kind="ExternalInput").ap()
    dlw_d = nc.dram_tensor("dlw", [128, 128], F16, kind="ExternalInput").ap()
    dle_d = nc.dram_tensor("dle", [128, 128], F16, kind="ExternalInput").ap()
    out_d = nc.dram_tensor("outp", [64, Q], F32, kind="ExternalOutput").ap()

    with tile.TileContext(nc) as tc:
        with (
            tc.tile_pool(name="big", bufs=1) as big,
            tc.tile_pool(name="wp", bufs=3) as wp,
            tc.tile_pool(name="scr", bufs=2) as scr,
            tc.tile_pool(name="small", bufs=1) as small,
            tc.tile_pool(name="stats", bufs=1) as stats,
            tc.tile_pool(name="ppool", bufs=2, space="PSUM") as ppool,
            tc.tile_pool(name="spool", bufs=2, space="PSUM") as spool,
            tc.tile_pool(name="fpool", bufs=2, space="PSUM") as fpool,
            tc.tile_pool(name="dram", bufs=1, space="DRAM") as dram,
        ):
            xall = small.tile([128, J * B], F16, tag="xall")
            dls = small.tile([128, 128], F16, tag="dls")
            dlw = small.tile([128, 128], F16, tag="dlw")
            dle = small.tile([128, 128], F16, tag="dle")
            nc.sync.dma_start(xall[:], xt_d)
            nc.sync.dma_start(dls[:], dls_d)
            nc.sync.dma_start(dlw[:], dlw_d)
            nc.sync.dma_start(dle[:], dle_d)

            xh = [big.tile([128, JC * Q], F16, tag=f"xh{g}", name=f"xh{g}")
                  for g in range(NCHUNK)]
            vv = small.tile([128, Q], F16, tag="vv", bufs=2)
            vvb = small.tile([128, 4 * Q], F16, tag="vvb")
            b1 = small.tile([128, J * OL], F16, tag="b1")
            ee = small.tile([128, J * OL], F32, tag="ee")
            cs = small.tile([128, J * OL], F16, tag="cs")
            den = small.tile([128, J], F32, tag="den")
            deng = small.tile([128, J], F32, tag="deng")
            ob = small.tile([64, Q], F32, tag="ob")
            ss = small.tile([128, Q], F32, tag="ss", bufs=2)

            for rep in range(repeat):
                # ---- production + dsum (iteration-0 i-sum), interleaved ----
                s0p = spool.tile([128, 2 * Q], F32, tag="sp")

                def dsum_wtile(w):
                    # 8 j's of wtile w -> 4 accumulating matmuls (2 j each)
                    g, half = (w * 8) // JC, (w * 8) % JC
                    src = xh[g][:, half * Q:(half + 8) * Q]
                    for m2 in range(4):
                        nc.tensor.matmul(
                            s0p[:, :],
                            dls[:],
                            src[:, m2 * 2 * Q:(m2 + 1) * 2 * Q],
                            start=(w == 0 and m2 == 0),
                            stop=(w == NWT - 1 and m2 == 3),
                        )

                for w in range(NWT):
                    wj = wp.tile([128, 8 * Q], F16, tag="w")
                    # W loads go on the Activation HWDGE ring: their pacing
                    # semaphores (wp buffer recycling) would otherwise
                    # head-of-line block the AR collective DMAs on the SP
                    # ring, serializing the softmax-denominator exchange.
                    nc.scalar.dma_start(wj[:], wt_d[w])
                    for hf in range(2):      # two psum tiles of 4 j each
                        pt = ppool.tile([128, 4 * Q], F32, tag="pt")
                        for jj in range(4):
                            j = w * 8 + hf * 4 + jj
                            for i2 in range(2):
                                # 2-quadrant packing: (0,0)+(64,64) pairs
                                # stream concurrently (cross-quadrant
                                # (0,64)/(64,0) faults at runtime on trn2)
                                pi = i2
                                sl = slice(i2 * 64, (i2 + 1) * 64)
                                ol = slice(pi * 64, (pi + 1) * 64)
                                nc.tensor.matmul(
                                    pt[ol, jj * Q:(jj + 1) * Q],
                                    xall[sl, j * B:(j + 1) * B],
                                    wj[sl, (hf * 4 + jj) * Q:
                                           (hf * 4 + jj + 1) * Q],
                                    start=True, stop=True,
                                    tile_position=(i2 * 64, pi * 64),
                                )
                        g = (w * 8) // JC
                        off = ((w * 8) % JC + hf * 4) * Q
                        dst = xh[g][:, off:off + 4 * Q]
                        # all copies on Scalar: keeps the DVE routing spine
                        # free (Tile would otherwise schedule these casts
                        # into the ws-pass AR-wait slots and overshoot)
                        nc.scalar.copy(dst, pt[:])
                    if w >= 3:
                        dsum_wtile(w - 3)
                for w in range(NWT - 3, NWT):
                    dsum_wtile(w)

                def psum_merge(sp, dst):
                    # sum the 2 j-parity psum partials into SBUF dst
                    # (a DVE op may read at most one PSUM operand)
                    nc.scalar.copy(dst[:], sp[:, Q:2 * Q])
                    nc.vector.tensor_add(dst[:], dst[:], sp[:, 0:Q])

                # ---- squash: factor f = n2/((1+n2)(n+eps)), n2 = sum_d s^2
                def squash(sv, out_ap, out_f32):
                    sq = stats.tile([128, Q], F32, tag="sq")
                    nc.vector.tensor_mul(sq[:], sv[:], sv[:])
                    n2 = stats.tile([128, OL], F32, tag="n2")
                    nc.vector.reduce_sum(
                        n2[:], sq.rearrange("p (d o) -> p o d", o=OL), axis=AX)
                    n1 = stats.tile([128, OL], F32, tag="n1")
                    nc.scalar.sqrt(n1[:], n2[:])
                    t1 = stats.tile([128, OL], F32, tag="t1")
                    nc.vector.tensor_scalar_add(t1[:], n2[:], 1.0)
                    nc.vector.reciprocal(t1[:], t1[:])
                    t2 = stats.tile([128, OL], F32, tag="t2")
                    nc.vector.tensor_scalar_add(t2[:], n1[:], EPS)
                    nc.vector.reciprocal(t2[:], t2[:])
                    ff = stats.tile([128, OL], F32, tag="ff")
                    nc.vector.tensor_mul(ff[:], n2[:], t1[:])
                    f2 = stats.tile([128, OL], F32, tag="f2")
                    nc.vector.tensor_mul(f2[:], ff[:], t2[:])
                    if out_f32:
                        nc.vector.tensor_tensor(
                            out_ap.rearrange("p (d o) -> p d o", o=OL),
                            sv[0:64, :].rearrange("p (d o) -> p d o", o=OL),
                            f2[0:64].unsqueeze(1).broadcast_to([64, D, OL]),
                            ALU.mult,
                        )
                    else:
                        f2h = stats.tile([128, OL], F16, tag="f2h")
                        nc.vector.tensor_copy(f2h[:], f2[:])
                        nc.vector.tensor_tensor(
                            out_ap.rearrange("p (d o) -> p d o", o=OL),
                            sv[:].rearrange("p (d o) -> p d o", o=OL),
                            f2h.unsqueeze(1).broadcast_to([128, D, OL]),
                            ALU.mult,
                        )

                # ---- one routing iteration ----------------------------------
                def agree_chunk(g, last_iter):
                    t = scr.tile([128, JC * Q], F16, tag="t")
                    nc.vector.tensor_tensor(
                        t[:],
                        xh[g][:],
                        vvb.unsqueeze(1).broadcast_to([128, 4, 4 * Q]),
                        ALU.mult,
                    )
                    # fold d 64 -> 8 on the PE: 8 accumulating matmuls
                    # (eye stationary), each one 8-wide d-block (32 q cols).
                    fp = fpool.tile([128, JC * 32], F32, tag="fp")
                    t4 = t.rearrange("p (j dB q) -> p j dB q", dB=8, q=32)
                    for dB in range(8):
                        nc.tensor.matmul(
                            fp[:], dle[:], t4[:, :, dB, :],
                            start=(dB == 0), stop=(dB == 7),
                        )
                    # residual fold 8 -> 1: scalar pulls psum, DVE adds
                    u = stats.tile([128, JC * 32], F16, tag="u")
                    nc.scalar.copy(u[:], fp[:])
                    u3 = u.rearrange("p (j x) -> p j x", x=32)
                    nc.vector.tensor_add(u3[:, :, 0:16],
                                         u3[:, :, 0:16], u3[:, :, 16:32])
                    nc.vector.tensor_add(u3[:, :, 0:8],
                                         u3[:, :, 0:8], u3[:, :, 8:16])
                    bsl = b1[:, g * JC * OL:(g + 1) * JC * OL]
                    b3 = bsl.rearrange("p (j o) -> p j o", o=OL)
                    if not last_iter:
                        nc.vector.tensor_add(b3, u3[:, :, 0:4],
                                             u3[:, :, 4:8])
                    else:
                        nc.vector.tensor_add(u3[:, :, 0:4], u3[:, :, 0:4],
                                             u3[:, :, 4:8])
                        nc.vector.tensor_add(b3, b3, u3[:, :, 0:4])

                def exp_den_kick2(a, b, ar_in, ar_out):
                    # softmax numerator + local denominator for the chunk
                    # segment [a, b), then kick the AllReduce of den.  The
                    # result pull (deng) is emitted separately (deng_pull2)
                    # so it doesn't head-of-line block later segments'
                    # input DMAs on the sync queue.
                    sl = slice(a * JC * OL, b * JC * OL)
                    dsl = slice(a * JC, b * JC)
                    nc.scalar.activation(ee[:, sl], b1[:, sl],
                                         mybir.ActivationFunctionType.Exp)
                    nc.vector.reduce_sum(
                        den[:, dsl],
                        ee[:, sl].rearrange("p (j o) -> p j o", o=OL),
                        axis=AX)
                    nc.sync.dma_start(ar_in[:], den[:, dsl])
                    nc.gpsimd.collective_compute(
                        "AllReduce",
                        ALU.add,
                        replica_groups=[list(range(CORES))],
                        ins=[ar_in.opt()],
                        outs=[ar_out.opt()],
                    )

                def deng_pull2(a, b, ar_out):
                    dsl = slice(a * JC, b * JC)
                    nc.sync.dma_start(deng[:, dsl], ar_out[:])

                def recip_c2(a, b):
                    nj = (b - a) * JC
                    dsl = slice(a * JC, b * JC)
                    sl = slice(a * JC * OL, b * JC * OL)
                    nc.vector.reciprocal_approx_fast(deng[:, dsl],
                                                     deng[:, dsl])
                    nc.vector.tensor_tensor(
                        cs[:, sl].rearrange("p (j o) -> p j o", o=OL),
                        ee[:, sl].rearrange("p (j o) -> p j o", o=OL),
                        deng[:, dsl].unsqueeze(2).broadcast_to(
                            [128, nj, OL]),
                        ALU.mult,
                    )

                def ws_chunk(g, sp):
                    xc = scr.tile([128, JC * Q], F16, tag="t")
                    nc.vector.tensor_tensor(
                        xc.rearrange("p (j d o) -> p j d o", d=D, o=OL),
                        xh[g].rearrange("p (j d o) -> p j d o", d=D, o=OL),
                        cs[:, g * JC * OL:(g + 1) * JC * OL]
                        .rearrange("p (j o) -> p j o", o=OL)
                        .unsqueeze(2).broadcast_to([128, JC, D, OL]),
                        ALU.mult,
                    )
                    for m2 in range(8):
                        nc.tensor.matmul(
                            sp[:, :],
                            dlw[:],
                            xc[:, m2 * 2 * Q:(m2 + 1) * 2 * Q],
                            start=(g == 0 and m2 == 0),
                            stop=(g == NCHUNK - 1 and m2 == 7),
                        )

                # ================= routing =================
                # iteration 0: c uniform (1/32 baked into dls) -> local s0
                s0s = ss  # alias (pool rotates by tag)
                psum_merge(s0p, s0s)
                if stage == 1:
                    nc.vector.tensor_copy(ob[:], s0s[0:64, :])
                    nc.sync.dma_start(out_d, ob[:])
                    continue
                def build_vvb():
                    # replicate vv 4x so the agree-mult broadcast AP has a
                    # 4-long outer loop instead of 16 (less DVE AP overhead)
                    nc.vector.tensor_copy(
                        vvb.rearrange("p (r q) -> p r q", q=Q),
                        vv.unsqueeze(1).broadcast_to([128, 4, Q]))

                squash(s0s, vv[:], out_f32=False)       # vv = out0
                build_vvb()
                if stage == 2:
                    nc.vector.tensor_copy(ob[:], vv[0:64, :])
                    nc.sync.dma_start(out_d, ob[:])
                    continue

                # it0: quarter segments (fully hidden behind remaining
                # agree chunks + ws1 start).  it1: three segments (6/5/5)
                # kicked at agree chunks 5/10/15 -- each arrives with
                # ~5us margin before its ws2 consumer, unlike halves
                # where the second half was break-even with consumption.
                SEGS = [[(0, 4), (4, 8), (8, 12), (12, 16)],
                        [(0, 6), (6, 11), (11, 16)]]
                ar_bufs = [[], []]
                for it in range(2):
                    for h, (a, b) in enumerate(SEGS[it]):
                        nj = (b - a) * JC
                        ai = dram.tile([128, nj], F32,
                                       tag=f"ari_{rep}_{it}_{h}",
                                       name=f"ari_{rep}_{it}_{h}")
                        ao = dram.tile([128, nj], F32,
                                       tag=f"aro_{rep}_{it}_{h}",
                                       name=f"aro_{rep}_{it}_{h}",
                                       addr_space="Shared")
                        ar_bufs[it].append((ai, ao))

                def iteration(it, sp, last):
                    segs = SEGS[it]
                    ends = {b - 1: i for i, (a, b) in enumerate(segs)}
                    starts = {a: i for i, (a, b) in enumerate(segs)}
                    for g in range(NCHUNK):
                        agree_chunk(g, last_iter=(it == 1))
                        if g in ends:
                            i = ends[g]
                            a, b = segs[i]
                            exp_den_kick2(a, b, *ar_bufs[it][i])
                    for g in range(NCHUNK):
                        if g in starts:
                            i = starts[g]
                            a, b = segs[i]
                            deng_pull2(a, b, ar_bufs[it][i][1])
                            recip_c2(a, b)
                        ws_chunk(g, sp)

                # iteration 1
                s1p = spool.tile([128, 2 * Q], F32, tag="sp")
                iteration(0, s1p, last=False)
                s1s = small.tile([128, Q], F32, tag="ss", bufs=2, name="s1s")
                psum_merge(s1p, s1s)
                if stage == 3:
                    nc.vector.tensor_copy(ob[:], b1[0:64, 0:Q])
                    nc.sync.dma_start(out_d, ob[:])
                    continue
                if stage == 4:
                    nc.vector.tensor_copy(ob[:], den[0:64, 0:J][:, 0:Q])
                    nc.sync.dma_start(out_d, ob[:])
                    continue
                if stage == 5:
                    nc.vector.tensor_copy(ob[:], cs[0:64, 0:Q])
                    nc.sync.dma_start(out_d, ob[:])
                    continue
                if stage == 6:
                    nc.vector.tensor_copy(ob[:], s1s[0:64, :])
                    nc.sync.dma_start(out_d, ob[:])
                    continue
                squash(s1s, vv[:], out_f32=False)       # vv = out1
                build_vvb()

                # iteration 2
                s2p = spool.tile([128, 2 * Q], F32, tag="sp")
                iteration(1, s2p, last=True)
                s2s = small.tile([128, Q], F32, tag="ss", bufs=2, name="s2s")
                psum_merge(s2p, s2s)
                squash(s2s, ob[:], out_f32=True)        # ob = out2 (f32)
                nc.sync.dma_start(out_d, ob[:])

    nc.compile()
    return nc


def _prep(x, weight):
    """Host-side shard + relayout + fp16 cast."""
    x16 = x.astype(np.float16)
    w16 = weight.astype(np.float16)
    # x: [B, I, M] -> [i2, m, j, b] -> [128, J*B]   (i = 2j + i2)
    xc = x16.reshape(B, J, 2, M).transpose(2, 3, 1, 0)
    xall = np.ascontiguousarray(xc.reshape(128, J * B))
    ws = []
    for c in range(CORES):
        wc = w16[c * OL:(c + 1) * OL]                 # [OL, I, D, M]
        # [o, j, i2, d, m] -> [j, i2, m, d, o]
        wc = wc.reshape(OL, J, 2, D, M).transpose(1, 2, 4, 3, 0)
        # group 8 j per wtile: [NWT, 8, 128, Q] -> [NWT, 128, 8*Q]
        wc = wc.reshape(NWT, 8, 128, Q).transpose(0, 2, 1, 3)
        ws.append(np.ascontiguousarray(wc.reshape(NWT, 128, 8 * Q)))
    kr = np.kron(np.ones((2, 2), np.float16), np.eye(64, dtype=np.float16))
    dls = kr / np.float16(32.0)
    dlw = kr
    dle = np.eye(128, dtype=np.float16)
    return xall, ws, dls, dlw, dle


class _Runner:
    """Compile once, execute many times (same as v1)."""

    def __init__(self, nc):
        import jax
        from jax.sharding import Mesh, PartitionSpec
        from jax.experimental.shard_map import shard_map
        from concourse import bass2jax
        from concourse.bass2jax import install_neuronx_cc_hook

        install_neuronx_cc_hook()
        self.nc = nc
        partition_name = (nc.partition_id_tensor.name
                          if nc.partition_id_tensor else None)
        in_names, out_names, out_avals, zero_outs = [], [], [], []
        for alloc in nc.m.functions[0].allocations:
            if not isinstance(alloc, mybir.MemoryLocationSet):
                continue
            name = alloc.memorylocations[0].name
            if alloc.kind == "ExternalInput":
                if name != partition_name:
                    in_names.append(name)
            elif alloc.kind == "ExternalOutput":
                out_names.append(name)
                shape = tuple(alloc.tensor_shape)
                dtype = mybir.dt.np(alloc.dtype)
                out_avals.append(jax.core.ShapedArray(shape, dtype))
                zero_outs.append(np.zeros(shape, dtype))
        n_params = len(in_names)
        n_outs = len(out_avals)
        all_in_names = list(in_names) + list(out_names)
        if partition_name is not None:
            all_in_names.append(partition_name)
        self.in_names = in_names
        self.out_names = out_names
        self.zero_outs = zero_outs
        self.out_avals = out_avals

        def _body(*args):
            operands = list(args)
            if partition_name is not None:
                operands.append(bass2jax.partition_id_tensor())
            outs = bass2jax._bass_exec_p.bind(
                *operands,
                out_avals=tuple(out_avals),
                in_names=tuple(all_in_names),
                out_names=tuple(out_names),
                lowering_input_output_aliases=(),
                sim_require_finite=True,
                sim_require_nnan=True,
                nc=nc,
            )
            return tuple(outs)

        devices = jax.devices()[:CORES]
        assert len(devices) == CORES
        mesh = Mesh(np.asarray(devices), ("core",))
        in_specs = (PartitionSpec("core"),) * (n_params + n_outs)
        out_specs = (PartitionSpec("core"),) * n_outs
        donate = tuple(range(n_params, n_params + n_outs))
        self.sharded = jax.jit(
            shard_map(_body, mesh=mesh, in_specs=in_specs,
                      out_specs=out_specs, check_rep=False),
            donate_argnums=donate, keep_unused=True,
        )

    def __call__(self, in_maps):
        concat_in = [
            np.concatenate([np.asarray(m[name]) for m in in_maps], axis=0)
            for name in self.in_names
        ]
        concat_zeros = [
            np.zeros((CORES * z.shape[0], *z.shape[1:]), z.dtype)
            for z in self.zero_outs
        ]
        out_arrs = self.sharded(*concat_in, *concat_zeros)
        return [
            {
                name: np.asarray(out_arrs[i]).reshape(
                    CORES, *self.out_avals[i].shape)[c]
                for i, name in enumerate(self.out_names)
            }
            for c in range(CORES)
        ]


_RUNNERS = {}


def _get_runner(repeat=1, stage=7):
    key = (repeat, stage)
    if key not in _RUNNERS:
        _RUNNERS[key] = _Runner(_build(repeat, stage))
    return _RUNNERS[key]


def make_in_maps(x, weight):
    xall, ws, dls, dlw, dle = _prep(np.asarray(x, np.float32),
                                    np.asarray(weight, np.float32))
    return [{"xt": xall, "wt": ws[c], "dls": dls, "dlw": dlw, "dle": dle}
            for c in range(CORES)]


def finish(results):
    # outp per core: [B=64, Q=256] f32 with q = d*OL + o_l
    parts = []
    for c in range(CORES):
        oc = results[c]["outp"].reshape(B, D, OL).transpose(0, 2, 1)
        parts.append(oc)                              # [B, OL, D]
    return np.ascontiguousarray(np.concatenate(parts, axis=1),
                                dtype=np.float32)     # [B, O, D]


def kernel(x, weight):
    runner = _get_runner()
    results = runner(make_in_maps(x, weight))
    return finish(results)


if __name__ == "__main__":
    rng = np.random.default_rng(0)
    x = rng.standard_normal((B, I, M)).astype(np.float32)
    w = (rng.standard_normal((O, I, D, M)) * 0.1).astype(np.float32)
    t0 = time.time()
    out = kernel(x, w)
    print("first call (incl compile):", time.time() - t0, "s; out", out.shape)

    # host reference
    xh = np.einsum("oidm,bim->boid", w.astype(np.float32), x)
    bb = np.zeros((B, O, I), np.float32)

    def squash_np(v):
        n = np.linalg.norm(v, axis=-1, keepdims=True)
        return (n * n / (1 + n * n)) * v / (n + EPS)

    for it in range(3):
        e = np.exp(bb - bb.max(axis=1, keepdims=True))
        c = e / e.sum(axis=1, keepdims=True)
        s = np.einsum("boi,boid->bod", c, xh)
        o = squash_np(s)
        if it < 2:
            bb = bb + np.einsum("boid,bod->boi", xh, o)
    err = np.abs(out - o).max() / (np.abs(o).max() + 1e-12)
    print("rel err vs host reference:", err)

